# revision 5
# baseline (speedup 1.0000x reference)
"""GumbelGraphNetworkClf fused Bass kernel for 8 trn2 NeuronCores (raw bass).

Math (per batch b):
  pre[i,j,:] = x[j]@W_e1[:D] + x[i]@W_e1[D:] + b_e1   (= A[j] + C[i])
  n2e = relu(pre); e2e = relu(n2e @ W_e2 + b_e2)
  agg[j,:] = sum_i adj[i,j] * e2e[i,j,:]
  out = log_softmax(nodeMLP(agg, x), axis=-1)

Per core: b = c//2, i-half = c%2 (256 local i rows, all 512 j).
Layout: features on partitions, j on free axis; software-pipelined i loop
with per-engine assignment (v2):
  PE    py[ho](it) = sum_t W2[t,ho].T @ n2e[t](it)   (4 matmuls, f16)
        agg[ho] += I @ msk[ho](it-L)                 (2 matmuls, lag L)
  ACT   e2e[ho](it) = relu(py[ho](it) + b2[ho])      (psum -> sbuf f16)
  DVE   n2e[t](it) = max(A[t] + C[t][:,it], 0)       (tensor_scalar f16)
        msk[0](it-2) = e2e[0] * abc                  (f16)
  POOL  msk[1](it-2) = e2e[1] * abc                  (f16)
The agg-ident matmuls trail the py matmuls by L iterations so the PE never
stalls on the ACT->DVE chain; ring-4 sbuf buffers provide the slack.
Then pair AllReduce of agg, node MLP on device, log_softmax over D=4 via
ones-matmul partition reduction (no max shift; logits are small here).
"""

import sys

sys.path.insert(0, "/opt/trn_rl_repo")

import numpy as np

import concourse.bass as bass
from concourse import mybir
from concourse.bass_utils import run_bass_kernel_spmd

B, N, D, H = 4, 512, 4, 256
import os
NI = int(os.environ.get("K_NI", "256"))
LAG = int(os.environ.get("K_LAG", "3"))
RING = 4
HT = 2
F16 = mybir.dt.float16
F32 = mybir.dt.float32
AF = mybir.ActivationFunctionType
OP = mybir.AluOpType

# f16 const pack (cols): W2 (2x256) | ident (128); A lives in its own
# tensor (af16) so DVE's A reads don't share SBUF banks with PE's W2 reads
O_W2 = 0
O_ID = O_W2 + 2 * H
CF16 = O_ID + 128
# f32 const pack (cols)
O_C = 0
O_B2 = O_C + 2 * NI
O_WN1 = O_B2 + 2
O_WN2 = O_WN1 + 2 * H
O_WO1H = O_WN2 + 2 * H
O_WO = O_WO1H + 2 * H
O_BN1 = O_WO + 2 * D
O_BN2 = O_BN1 + 2
O_BO1 = O_BN2 + 2
O_XT = O_BO1 + 2          # rows 0-3
O_WO1X = O_XT + N         # rows 0-3
O_BO = O_WO1X + H         # rows 0-3
O_ONES4 = O_BO + 1        # [4,1] ones, rows 0-3
O_ONES14 = O_ONES4 + 1    # [1,4] ones, row 0
CF32 = O_ONES14 + 4

_CACHE = {}


def build_program():
    nc = bass.Bass("TRN2", target_bir_lowering=False, num_devices=8)

    cf16_ext = nc.dram_tensor("cf16", [128, CF16], F16, kind="ExternalInput")
    af16_ext = nc.dram_tensor("af16", [128, 2 * N], F16, kind="ExternalInput")
    cf32_ext = nc.dram_tensor("cf32", [128, CF32], F32, kind="ExternalInput")
    adj_ext = nc.dram_tensor("adjr", [NI, N], F16, kind="ExternalInput")
    out_ext = nc.dram_tensor("out", [N, D], F32, kind="ExternalOutput")
    aggd = nc.dram_tensor("aggd", [H, N], F32)
    aggr = nc.dram_tensor("aggr", [H, N], F32)

    # ---- milestone tables (mirror the emission order below) ----
    # p_sem: PE loop stream: per it [py x2 incs], then (it>=LAG) [agg(it-LAG) x2]
    p_py, p_agg = {}, {}
    p = 0
    for it in range(NI):
        p += 2; p_py[it] = p
        jt = it - LAG
        if jt >= 0:
            p += 2; p_agg[jt] = p
    for jt in range(NI - LAG, NI):
        p += 2; p_agg[jt] = p
    P_LOOP = p
    # a_sem: ACT loop: per it [relu0, relu1]
    a_relu0, a_relu1 = {}, {}
    a = 0
    for it in range(NI):
        a += 1; a_relu0[it] = a
        a += 1; a_relu1[it] = a
    # v_sem: DVE loop: per s [ts0, ts1], then (s>=2) [msk0(s-2)]; tail 2 msk0,
    # then 2 agg copies
    v_n2e, v_msk0 = {}, {}
    v = 0
    for s in range(NI):
        v += 2; v_n2e[s] = v
        j = s - 2
        if j >= 0:
            v += 1; v_msk0[j] = v
    for j in (NI - 2, NI - 1):
        v += 1; v_msk0[j] = v
    v += 2
    V_AGGCOPY = v
    # g_sem: POOL loop: per j [msk1]
    g_msk1 = {j: j + 1 for j in range(NI)}
    # d_sem milestones
    D_CONST = 48

    def d_abc(it):
        return D_CONST + 16 * (it // 8 + 1)

    D_LOOP = d_abc(NI - 1)
    D_AGGD = D_LOOP + 32
    D_AGGR = D_AGGD + 32

    from contextlib import ExitStack
    with ExitStack() as ctx:
        e = ctx.enter_context
        cf16 = e(nc.sbuf_tensor([128, CF16], F16))
        cf32 = e(nc.sbuf_tensor([128, CF32], F32))
        abc = [e(nc.sbuf_tensor(f"abc{k}", [128, 8 * N], F16)) for k in range(2)]
        n2e = [[e(nc.sbuf_tensor(f"n2e{t}{k}", [128, N], F16)) for k in range(RING)] for t in range(2)]
        e2e = [[e(nc.sbuf_tensor(f"e2e{t}{k}", [128, N], F16)) for k in range(RING)] for t in range(2)]
        msk = [[e(nc.sbuf_tensor(f"msk{t}{k}", [128, N], F16)) for k in range(RING)] for t in range(2)]
        a_sb = e(nc.sbuf_tensor("asb16", [128, 2 * N], F16))
        agg_sb = [e(nc.sbuf_tensor(f"aggs{k}", [128, N], F32)) for k in range(2)]
        out1 = [e(nc.sbuf_tensor(f"out1{k}", [128, N], F32)) for k in range(2)]
        out2 = [e(nc.sbuf_tensor(f"out2{k}", [128, N], F32)) for k in range(2)]
        out4 = [e(nc.sbuf_tensor(f"out4{k}", [128, N], F32)) for k in range(2)]
        out5 = e(nc.sbuf_tensor([4, N], F32))
        ex = e(nc.sbuf_tensor([4, N], F32))
        ls = e(nc.sbuf_tensor([1, N], F32))
        res = e(nc.sbuf_tensor([4, N], F32))
        py00 = e(nc.psum_tensor([128, N], F32))
        py01 = e(nc.psum_tensor([128, N], F32))
        py10 = e(nc.psum_tensor([128, N], F32))
        py11 = e(nc.psum_tensor([128, N], F32))
        agg0 = e(nc.psum_tensor([128, N], F32))
        agg1 = e(nc.psum_tensor([128, N], F32))
        d_sem = e(nc.semaphore("d_sem"))
        v_sem = e(nc.semaphore("v_sem"))
        p_sem = e(nc.semaphore("p_sem"))
        a_sem = e(nc.semaphore("a_sem"))
        g_sem = e(nc.semaphore("g_sem"))
        cc_sem = e(nc.semaphore("cc_sem"))
        pp_sem = e(nc.semaphore("pp_sem"))
        aa_sem = e(nc.semaphore("aa_sem"))
        vv_sem = e(nc.semaphore("vv_sem"))
        block = e(nc.Block())
        py = [[py00, py01], [py10, py11]]
        agg_ps = [agg0, agg1]
        A_sb = [a_sb[:, t * N : (t + 1) * N] for t in range(HT)]
        W2_sb = [cf16[:, O_W2 + t * H : O_W2 + (t + 1) * H] for t in range(HT)]
        ident = cf16[:, O_ID : O_ID + 128]
        C_sb = [cf32[:, O_C + t * NI : O_C + (t + 1) * NI] for t in range(HT)]
        b2_sb = [cf32[:, O_B2 + t : O_B2 + t + 1] for t in range(HT)]
        Wn1_sb = [cf32[:, O_WN1 + t * H : O_WN1 + (t + 1) * H] for t in range(HT)]
        Wn2_sb = [cf32[:, O_WN2 + t * H : O_WN2 + (t + 1) * H] for t in range(HT)]
        Wo1h_sb = [cf32[:, O_WO1H + t * H : O_WO1H + (t + 1) * H] for t in range(HT)]
        Wo_sb = [cf32[:, O_WO + t * D : O_WO + (t + 1) * D] for t in range(HT)]
        bn1_sb = [cf32[:, O_BN1 + t : O_BN1 + t + 1] for t in range(HT)]
        bn2_sb = [cf32[:, O_BN2 + t : O_BN2 + t + 1] for t in range(HT)]
        bo1_sb = [cf32[:, O_BO1 + t : O_BO1 + t + 1] for t in range(HT)]
        xT_sb = cf32[0:D, O_XT : O_XT + N]
        Wo1x_sb = cf32[0:D, O_WO1X : O_WO1X + H]
        bo_sb = cf32[0:D, O_BO : O_BO + 1]
        ones4 = cf32[0:D, O_ONES4 : O_ONES4 + 1]
        ones14 = cf32[0:1, O_ONES14 : O_ONES14 + 4]
        # post-loop PSUM bank reuse
        ps_mlp = [py00, py10]
        ps5 = agg0[0:4, :]
        ps_sum = py01[0:1, :]
        ps_ls4 = py11[0:4, :]

        def abc_ap(j):
            return abc[(j // 8) % 2][:, (j % 8) * N : (j % 8 + 1) * N]

        @block.sync
        def _(sync):
            sync.dma_start(cf16[:], cf16_ext[:, :]).then_inc(d_sem, 16)
            sync.dma_start(cf32[:], cf32_ext[:, :]).then_inc(d_sem, 16)
            sync.dma_start(a_sb[:], af16_ext[:, :]).then_inc(d_sem, 16)
            for k in range(NI // 8):
                if k >= 2:
                    jlast = 8 * (k - 2) + 7          # abc[k%2] WAR
                    sync.wait_ge(v_sem, v_msk0[jlast])
                    sync.wait_ge(g_sem, g_msk1[jlast])
                sync.dma_start(
                    abc[k % 2][:],
                    adj_ext[None, 8 * k : 8 * (k + 1), :].broadcast_to([128, 8, N]),
                ).then_inc(d_sem, 16)
            sync.wait_ge(v_sem, V_AGGCOPY)            # agg_sb written
            for t in range(HT):
                sync.dma_start(aggd[t * 128 : (t + 1) * 128, :], agg_sb[t][:]).then_inc(d_sem, 16)
            sync.wait_ge(cc_sem, 1)
            for t in range(HT):
                sync.dma_start(agg_sb[t][:], aggr[t * 128 : (t + 1) * 128, :]).then_inc(d_sem, 16)
            sync.wait_ge(vv_sem, 1)
            with nc.allow_non_contiguous_dma(reason="8KB total, once"):
                for d in range(D):
                    sync.dma_start(out_ext[:, d : d + 1], res[d : d + 1, :]).then_inc(d_sem, 16)

        @block.gpsimd
        def _(gpsimd):
            for j in range(NI):
                gpsimd.wait_ge(a_sem, a_relu1[j])
                gpsimd.wait_ge(d_sem, d_abc(j))
                if j >= RING:
                    gpsimd.wait_ge(p_sem, p_agg[j - RING])   # msk1 slot WAR
                nc.gpsimd.tensor_mul(
                    msk[1][j % RING][:], e2e[1][j % RING][:], abc_ap(j)
                ).then_inc(g_sem, 1)
            gpsimd.wait_ge(d_sem, D_AGGD)
            nc.gpsimd.collective_compute(
                "AllReduce", OP.add,
                replica_groups=[[0, 1], [2, 3], [4, 5], [6, 7]],
                ins=[aggd[:]], outs=[aggr[:]],
            ).then_inc(cc_sem, 1)

        @block.vector
        def _(vector):
            for s in range(NI):
                if s == 0:
                    vector.wait_ge(d_sem, D_CONST)
                if s >= RING:
                    vector.wait_ge(p_sem, p_py[s - RING])    # n2e slot WAR
                for t in range(HT):
                    nc.vector.tensor_scalar(
                        n2e[t][s % RING][:], A_sb[t], C_sb[t][:, s : s + 1], 0.0,
                        op0=OP.add, op1=OP.max,
                    ).then_inc(v_sem, 1)
                j = s - 2
                if j >= 0:
                    vector.wait_ge(a_sem, a_relu0[j])
                    vector.wait_ge(d_sem, d_abc(j))
                    if j >= RING:
                        vector.wait_ge(p_sem, p_agg[j - RING])   # msk0 slot WAR
                    nc.vector.tensor_mul(
                        msk[0][j % RING][:], e2e[0][j % RING][:], abc_ap(j)
                    ).then_inc(v_sem, 1)
            for j in (NI - 2, NI - 1):
                vector.wait_ge(a_sem, a_relu0[j])
                vector.wait_ge(d_sem, d_abc(j))
                vector.wait_ge(p_sem, p_agg[j - RING])       # msk0 slot WAR
                nc.vector.tensor_mul(
                    msk[0][j % RING][:], e2e[0][j % RING][:], abc_ap(j)
                ).then_inc(v_sem, 1)
            vector.wait_ge(p_sem, P_LOOP)                    # all agg done
            for t in range(HT):
                nc.vector.tensor_copy(agg_sb[t][:], agg_ps[t][:]).then_inc(v_sem, 1)
            vector.wait_ge(pp_sem, 9)
            nc.vector.tensor_tensor(res[:], out5[:], ps_ls4, op=OP.subtract).then_inc(vv_sem, 1)

        @block.scalar
        def _(scalar):
            for it in range(NI):
                q = it % 2
                scalar.wait_ge(p_sem, p_py[it])
                for ho in range(HT):
                    if it >= RING:
                        # e2e slot WAR: msk op of it-RING consumed it
                        if ho == 0:
                            scalar.wait_ge(v_sem, v_msk0[it - RING])
                        else:
                            scalar.wait_ge(g_sem, g_msk1[it - RING])
                    nc.scalar.activation(
                        e2e[ho][it % RING][:], py[ho][q][:], AF.Relu, bias=b2_sb[ho]
                    ).then_inc(a_sem, 1)
            # ---- node MLP activations ----
            for ho in range(HT):
                scalar.wait_ge(pp_sem, ho + 1)
                nc.scalar.activation(out1[ho][:], ps_mlp[ho][:], AF.Relu,
                                     bias=bn1_sb[ho]).then_inc(aa_sem, 1)
            for ho in range(HT):
                scalar.wait_ge(pp_sem, 2 + ho + 1)
                nc.scalar.activation(out2[ho][:], ps_mlp[ho][:], AF.Relu,
                                     bias=bn2_sb[ho]).then_inc(aa_sem, 1)
            for ho in range(HT):
                scalar.wait_ge(pp_sem, 4 + ho + 1)
                nc.scalar.activation(out4[ho][:], ps_mlp[ho][:], AF.Identity,
                                     bias=bo1_sb[ho]).then_inc(aa_sem, 1)
            scalar.wait_ge(pp_sem, 7)
            nc.scalar.activation(out5[:], ps5, AF.Identity, bias=bo_sb).then_inc(aa_sem, 1)
            nc.scalar.activation(ex[:], out5[:], AF.Exp).then_inc(aa_sem, 1)
            scalar.wait_ge(pp_sem, 8)
            nc.scalar.activation(ls[:], ps_sum, AF.Ln).then_inc(aa_sem, 1)

        @block.tensor
        def _(pe):
            def agg_mm(jt):
                pe.wait_ge(v_sem, v_msk0[jt])
                pe.wait_ge(g_sem, g_msk1[jt])
                for ho in range(HT):
                    nc.tensor.matmul(
                        agg_ps[ho][:], ident, msk[ho][jt % RING][:],
                        start=(jt == 0), stop=(jt == NI - 1),
                    ).then_inc(p_sem, 1)

            for it in range(NI):
                q = it % 2
                pe.wait_ge(v_sem, v_n2e[it])                # n2e(it) ready
                if it >= 2:
                    pe.wait_ge(a_sem, a_relu1[it - 2])      # py[q] WAR
                for ho in range(HT):
                    nc.tensor.matmul(
                        py[ho][q][:], W2_sb[0][:, ho * 128 : (ho + 1) * 128],
                        n2e[0][it % RING][:], start=True, stop=False,
                    )
                    nc.tensor.matmul(
                        py[ho][q][:], W2_sb[1][:, ho * 128 : (ho + 1) * 128],
                        n2e[1][it % RING][:], start=False, stop=True,
                    ).then_inc(p_sem, 1)
                if it >= LAG:
                    agg_mm(it - LAG)
            for jt in range(NI - LAG, NI):
                agg_mm(jt)
            # ---- node MLP matmuls ----  pp_sem milestones:
            # 1,2: mlp1[ho]  3,4: mlp2[ho]  5,6: out4 ps[ho]  7: ps5
            # 8: ps_sum  9: ps_ls4
            pe.wait_ge(d_sem, D_AGGR)
            for ho in range(HT):
                for t in range(HT):
                    mm = nc.tensor.matmul(
                        ps_mlp[ho][:], Wn1_sb[t][:, ho * 128 : (ho + 1) * 128],
                        agg_sb[t][:], start=(t == 0), stop=(t == 1),
                    )
                mm.then_inc(pp_sem, 1)
            for ho in range(HT):
                pe.wait_ge(aa_sem, 2)
                for t in range(HT):
                    mm = nc.tensor.matmul(
                        ps_mlp[ho][:], Wn2_sb[t][:, ho * 128 : (ho + 1) * 128],
                        out1[t][:], start=(t == 0), stop=(t == 1),
                    )
                mm.then_inc(pp_sem, 1)
            for ho in range(HT):
                pe.wait_ge(aa_sem, 4)
                nc.tensor.matmul(
                    ps_mlp[ho][:], Wo1x_sb[:, ho * 128 : (ho + 1) * 128], xT_sb,
                    start=True, stop=False,
                )
                for t in range(HT):
                    mm = nc.tensor.matmul(
                        ps_mlp[ho][:], Wo1h_sb[t][:, ho * 128 : (ho + 1) * 128],
                        out2[t][:], start=False, stop=(t == 1),
                    )
                mm.then_inc(pp_sem, 1)
            pe.wait_ge(aa_sem, 6)
            for t in range(HT):
                mm = nc.tensor.matmul(
                    ps5, Wo_sb[t], out4[t][:], start=(t == 0), stop=(t == 1),
                )
            mm.then_inc(pp_sem, 1)
            pe.wait_ge(aa_sem, 8)                           # ex ready
            nc.tensor.matmul(ps_sum, ones4, ex[:], start=True, stop=True).then_inc(pp_sem, 1)
            pe.wait_ge(aa_sem, 9)                           # ls ready
            nc.tensor.matmul(ps_ls4, ones14, ls[:], start=True, stop=True).then_inc(pp_sem, 1)

    return nc


def make_in_maps(x, adj, W_e1, b_e1, W_e2, b_e2, W_n1, b_n1, W_n2, b_n2,
                 W_o1, b_o1, W_o, b_o):
    in_maps = []
    for c in range(8):
        b = c // 2
        i0 = (c % 2) * NI
        A_full = x[b] @ W_e1[:D]                     # [N, H]
        C_full = x[b] @ W_e1[D:] + b_e1              # [N, H]

        cf16 = np.zeros((128, CF16), np.float16)
        AT = A_full.T.astype(np.float16)             # [H, N]
        af16 = np.zeros((128, 2 * N), np.float16)
        for t in range(HT):
            af16[:, t * N : (t + 1) * N] = AT[t * 128 : (t + 1) * 128]
            cf16[:, O_W2 + t * H : O_W2 + (t + 1) * H] = \
                W_e2[t * 128 : (t + 1) * 128].astype(np.float16)
        cf16[:, O_ID : O_ID + 128] = np.eye(128, dtype=np.float16)

        cf32 = np.zeros((128, CF32), np.float32)
        CT = C_full[i0 : i0 + NI].T.astype(np.float32)   # [H, NI]
        for t in range(HT):
            r = slice(t * 128, (t + 1) * 128)
            cf32[:, O_C + t * NI : O_C + (t + 1) * NI] = CT[r]
            cf32[:, O_B2 + t] = b_e2[r]
            cf32[:, O_WN1 + t * H : O_WN1 + (t + 1) * H] = W_n1[r]
            cf32[:, O_WN2 + t * H : O_WN2 + (t + 1) * H] = W_n2[r]
            cf32[:, O_WO1H + t * H : O_WO1H + (t + 1) * H] = W_o1[D:][r]
            cf32[:, O_WO + t * D : O_WO + (t + 1) * D] = W_o[r]
            cf32[:, O_BN1 + t] = b_n1[r]
            cf32[:, O_BN2 + t] = b_n2[r]
            cf32[:, O_BO1 + t] = b_o1[r]
        cf32[0:D, O_XT : O_XT + N] = x[b].T
        cf32[0:D, O_WO1X : O_WO1X + H] = W_o1[:D]
        cf32[0:D, O_BO] = b_o
        cf32[0:D, O_ONES4] = 1.0
        cf32[0:1, O_ONES14 : O_ONES14 + 4] = 1.0

        in_maps.append({
            "cf16": cf16,
            "af16": af16,
            "cf32": cf32,
            "adjr": adj[b, i0 : i0 + NI, :].astype(np.float16),
        })
    return in_maps


def run(trace=False, **inputs):
    if "nc" not in _CACHE:
        _CACHE["nc"] = build_program()
    nc = _CACHE["nc"]
    in_maps = make_in_maps(**{k: np.asarray(v) for k, v in inputs.items()})
    r = run_bass_kernel_spmd(nc, in_maps, list(range(8)), trace=trace)
    out = np.stack([r.results[2 * b]["out"] for b in range(B)]).astype(np.float32)
    return out, r


def kernel(**inputs):
    out, _ = run(trace=False, **inputs)
    return out


# revision 6
# speedup vs baseline: 1.4187x; 1.4187x over previous
"""GumbelGraphNetworkClf fused Bass kernel for 8 trn2 NeuronCores (raw bass).

Math (per batch b):
  pre[i,j,:] = x[j]@W_e1[:D] + x[i]@W_e1[D:] + b_e1   (= A[j] + C[i])
  n2e = relu(pre); e2e = relu(n2e @ W_e2 + b_e2)
  agg[j,:] = sum_i adj[i,j] * e2e[i,j,:]
  out = log_softmax(nodeMLP(agg, x), axis=-1)

Per core: b = c//2, i-half = c%2 (256 local i rows, all 512 j).
Layout: features on partitions, j on free axis; software-pipelined i loop
with per-engine assignment (v3):
  PE    py[ho](it) = sum_t W2[t,ho].T @ n2e[t](it)   (4 matmuls, f16)
        agg[ho] += I @ msk[ho](it-L)                 (2 matmuls, lag L=4)
  ACT   e2e[ho](it) = relu(py[ho](it) + b2[ho])      (psum -> sbuf f16)
  DVE   n2e[t](it) = max(A[t] + C[t][:,it], 0)       (tensor_scalar f16)
        msk[ho](pair) = e2e[ho](pair) * abc(pair)    ([128,1024] merged TT)
The msk multiplies are merged over iteration pairs (e2e/msk are single
ring tensors so pair slots are contiguous), and the gpsimd engine is kept
out of the loop entirely: its software (Q7) tensor ops starve the DVE's
SBUF ports, stretching DVE ops ~3.5x when they overlap.
The agg-ident matmuls trail the py matmuls by L iterations so the PE never
stalls on the ACT->DVE chain; ring-4 buffers provide the slack.
Then pair AllReduce of agg, node MLP on device, log_softmax over D=4 via
ones-matmul partition reduction (no max shift; logits are small here).
"""

import sys

sys.path.insert(0, "/opt/trn_rl_repo")

import numpy as np

import concourse.bass as bass
from concourse import mybir
from concourse.bass_utils import run_bass_kernel_spmd

B, N, D, H = 4, 512, 4, 256
import os
NI = int(os.environ.get("K_NI", "256"))
LAG = int(os.environ.get("K_LAG", "4"))
RING = 4
HT = 2
F16 = mybir.dt.float16
F32 = mybir.dt.float32
AF = mybir.ActivationFunctionType
OP = mybir.AluOpType

# f16 const pack (cols): W2 (2x256) | ident (128)
O_W2 = 0
O_ID = O_W2 + 2 * H
CF16 = O_ID + 128
# f32 const pack (cols)
O_C = 0
O_B2 = O_C + 2 * NI
O_WN1 = O_B2 + 2
O_WN2 = O_WN1 + 2 * H
O_WO1H = O_WN2 + 2 * H
O_WO = O_WO1H + 2 * H
O_BN1 = O_WO + 2 * D
O_BN2 = O_BN1 + 2
O_BO1 = O_BN2 + 2
O_XT = O_BO1 + 2          # rows 0-3
O_WO1X = O_XT + N         # rows 0-3
O_BO = O_WO1X + H         # rows 0-3
O_ONES4 = O_BO + 1        # [4,1] ones, rows 0-3
O_ONES14 = O_ONES4 + 1    # [1,4] ones, row 0
CF32 = O_ONES14 + 4

NP = NI // 2              # iteration pairs

_CACHE = {}


def build_program():
    nc = bass.Bass("TRN2", target_bir_lowering=False, num_devices=8)

    cf16_ext = nc.dram_tensor("cf16", [128, CF16], F16, kind="ExternalInput")
    af16_ext = nc.dram_tensor("af16", [128, 2 * N], F16, kind="ExternalInput")
    cf32_ext = nc.dram_tensor("cf32", [128, CF32], F32, kind="ExternalInput")
    adj_ext = nc.dram_tensor("adjr", [NI, N], F16, kind="ExternalInput")
    out_ext = nc.dram_tensor("out", [N, D], F32, kind="ExternalOutput")
    aggd = nc.dram_tensor("aggd", [H, N], F32)
    aggr = nc.dram_tensor("aggr", [H, N], F32)

    # ---- milestone tables (mirror the emission order below) ----
    # p_sem: PE loop stream: per it [py x2 incs], then (it>=LAG) [agg(it-LAG) x2]
    p_py, p_agg = {}, {}
    p = 0
    for it in range(NI):
        p += 2; p_py[it] = p
        jt = it - LAG
        if jt >= 0:
            p += 2; p_agg[jt] = p
    for jt in range(NI - LAG, NI):
        p += 2; p_agg[jt] = p
    P_LOOP = p
    # a_sem: ACT loop: per it [relu0, relu1]
    a_relu0, a_relu1 = {}, {}
    a = 0
    for it in range(NI):
        a += 1; a_relu0[it] = a
        a += 1; a_relu1[it] = a
    # v_sem: DVE loop: per s [ts0, ts1]; odd s with pr=(s-3)//2 >= 0:
    # [ttm0(pr), ttm1(pr)]; tail [ttm0(NP-1), ttm1(NP-1)], 2 agg copies
    v_n2e, v_mskp0, v_mskp1 = {}, {}, {}
    v = 0
    for s in range(NI):
        v += 2; v_n2e[s] = v
        if s % 2 == 1:
            pr = (s - 3) // 2
            if pr >= 0:
                v += 1; v_mskp0[pr] = v
                v += 1; v_mskp1[pr] = v
    v += 1; v_mskp0[NP - 1] = v
    v += 1; v_mskp1[NP - 1] = v
    v += 2
    V_AGGCOPY = v
    # d_sem milestones
    D_CONST = 48

    def d_abc(it):
        return D_CONST + 16 * (it // 8 + 1)

    D_LOOP = d_abc(NI - 1)
    D_AGGD = D_LOOP + 32
    D_AGGR = D_AGGD + 32

    from contextlib import ExitStack
    with ExitStack() as ctx:
        e = ctx.enter_context
        cf16 = e(nc.sbuf_tensor([128, CF16], F16))
        cf32 = e(nc.sbuf_tensor([128, CF32], F32))
        abc = [e(nc.sbuf_tensor(f"abc{k}", [128, 8 * N], F16)) for k in range(2)]
        n2e = [[e(nc.sbuf_tensor(f"n2e{t}{k}", [128, N], F16)) for k in range(RING)] for t in range(2)]
        e2e = [e(nc.sbuf_tensor(f"e2e{t}", [128, RING * N], F16)) for t in range(2)]
        msk = [e(nc.sbuf_tensor(f"msk{t}", [128, RING * N], F16)) for t in range(2)]
        a_sb = e(nc.sbuf_tensor("asb16", [128, 2 * N], F16))
        agg_sb = [e(nc.sbuf_tensor(f"aggs{k}", [128, N], F32)) for k in range(2)]
        out1 = [e(nc.sbuf_tensor(f"out1{k}", [128, N], F32)) for k in range(2)]
        out2 = [e(nc.sbuf_tensor(f"out2{k}", [128, N], F32)) for k in range(2)]
        out4 = [e(nc.sbuf_tensor(f"out4{k}", [128, N], F32)) for k in range(2)]
        out5 = e(nc.sbuf_tensor([4, N], F32))
        ex = e(nc.sbuf_tensor([4, N], F32))
        ls = e(nc.sbuf_tensor([1, N], F32))
        res = e(nc.sbuf_tensor([4, N], F32))
        py00 = e(nc.psum_tensor([128, N], F32))
        py01 = e(nc.psum_tensor([128, N], F32))
        py10 = e(nc.psum_tensor([128, N], F32))
        py11 = e(nc.psum_tensor([128, N], F32))
        agg0 = e(nc.psum_tensor([128, N], F32))
        agg1 = e(nc.psum_tensor([128, N], F32))
        d_sem = e(nc.semaphore("d_sem"))
        v_sem = e(nc.semaphore("v_sem"))
        p_sem = e(nc.semaphore("p_sem"))
        a_sem = e(nc.semaphore("a_sem"))
        cc_sem = e(nc.semaphore("cc_sem"))
        pp_sem = e(nc.semaphore("pp_sem"))
        aa_sem = e(nc.semaphore("aa_sem"))
        vv_sem = e(nc.semaphore("vv_sem"))
        block = e(nc.Block())
        py = [[py00, py01], [py10, py11]]
        agg_ps = [agg0, agg1]
        A_sb = [a_sb[:, t * N : (t + 1) * N] for t in range(HT)]
        W2_sb = [cf16[:, O_W2 + t * H : O_W2 + (t + 1) * H] for t in range(HT)]
        ident = cf16[:, O_ID : O_ID + 128]
        C_sb = [cf32[:, O_C + t * NI : O_C + (t + 1) * NI] for t in range(HT)]
        b2_sb = [cf32[:, O_B2 + t : O_B2 + t + 1] for t in range(HT)]
        Wn1_sb = [cf32[:, O_WN1 + t * H : O_WN1 + (t + 1) * H] for t in range(HT)]
        Wn2_sb = [cf32[:, O_WN2 + t * H : O_WN2 + (t + 1) * H] for t in range(HT)]
        Wo1h_sb = [cf32[:, O_WO1H + t * H : O_WO1H + (t + 1) * H] for t in range(HT)]
        Wo_sb = [cf32[:, O_WO + t * D : O_WO + (t + 1) * D] for t in range(HT)]
        bn1_sb = [cf32[:, O_BN1 + t : O_BN1 + t + 1] for t in range(HT)]
        bn2_sb = [cf32[:, O_BN2 + t : O_BN2 + t + 1] for t in range(HT)]
        bo1_sb = [cf32[:, O_BO1 + t : O_BO1 + t + 1] for t in range(HT)]
        xT_sb = cf32[0:D, O_XT : O_XT + N]
        Wo1x_sb = cf32[0:D, O_WO1X : O_WO1X + H]
        bo_sb = cf32[0:D, O_BO : O_BO + 1]
        ones4 = cf32[0:D, O_ONES4 : O_ONES4 + 1]
        ones14 = cf32[0:1, O_ONES14 : O_ONES14 + 4]
        # post-loop PSUM bank reuse
        ps_mlp = [py00, py10]
        ps5 = agg0[0:4, :]
        ps_sum = py01[0:1, :]
        ps_ls4 = py11[0:4, :]

        def e2e_ap(ho, it):
            return e2e[ho][:, (it % RING) * N : (it % RING + 1) * N]

        def msk_ap(ho, it):
            return msk[ho][:, (it % RING) * N : (it % RING + 1) * N]

        def e2e_pair_ap(ho, pr):
            r = (2 * pr) % RING
            return e2e[ho][:, r * N : (r + 2) * N]

        def msk_pair_ap(ho, pr):
            r = (2 * pr) % RING
            return msk[ho][:, r * N : (r + 2) * N]

        def abc_pair_ap(pr):
            j = 2 * pr
            return abc[(j // 8) % 2][:, (j % 8) * N : (j % 8 + 2) * N]

        def emit_ttm(vector, pr):
            vector.wait_ge(a_sem, a_relu0[2 * pr + 1])
            vector.wait_ge(d_sem, d_abc(2 * pr))
            if pr >= 2:
                vector.wait_ge(p_sem, p_agg[2 * (pr - 2) + 1])  # msk slot WAR
            nc.vector.tensor_mul(
                msk_pair_ap(0, pr), e2e_pair_ap(0, pr), abc_pair_ap(pr)
            ).then_inc(v_sem, 1)
            vector.wait_ge(a_sem, a_relu1[2 * pr + 1])
            nc.vector.tensor_mul(
                msk_pair_ap(1, pr), e2e_pair_ap(1, pr), abc_pair_ap(pr)
            ).then_inc(v_sem, 1)

        @block.sync
        def _(sync):
            sync.dma_start(cf16[:], cf16_ext[:, :]).then_inc(d_sem, 16)
            sync.dma_start(cf32[:], cf32_ext[:, :]).then_inc(d_sem, 16)
            sync.dma_start(a_sb[:], af16_ext[:, :]).then_inc(d_sem, 16)
            for k in range(NI // 8):
                if k >= 2:
                    sync.wait_ge(v_sem, v_mskp1[4 * (k - 2) + 3])  # abc[k%2] WAR
                sync.dma_start(
                    abc[k % 2][:],
                    adj_ext[None, 8 * k : 8 * (k + 1), :].broadcast_to([128, 8, N]),
                ).then_inc(d_sem, 16)
            sync.wait_ge(v_sem, V_AGGCOPY)            # agg_sb written
            for t in range(HT):
                sync.dma_start(aggd[t * 128 : (t + 1) * 128, :], agg_sb[t][:]).then_inc(d_sem, 16)
            sync.wait_ge(cc_sem, 1)
            for t in range(HT):
                sync.dma_start(agg_sb[t][:], aggr[t * 128 : (t + 1) * 128, :]).then_inc(d_sem, 16)
            sync.wait_ge(vv_sem, 1)
            with nc.allow_non_contiguous_dma(reason="8KB total, once"):
                for d in range(D):
                    sync.dma_start(out_ext[:, d : d + 1], res[d : d + 1, :]).then_inc(d_sem, 16)

        @block.gpsimd
        def _(gpsimd):
            gpsimd.wait_ge(d_sem, D_AGGD)
            nc.gpsimd.collective_compute(
                "AllReduce", OP.add,
                replica_groups=[[0, 1], [2, 3], [4, 5], [6, 7]],
                ins=[aggd[:]], outs=[aggr[:]],
            ).then_inc(cc_sem, 1)

        @block.vector
        def _(vector):
            for s in range(NI):
                if s == 0:
                    vector.wait_ge(d_sem, D_CONST)
                if s >= RING:
                    vector.wait_ge(p_sem, p_py[s - RING])    # n2e slot WAR
                for t in range(HT):
                    nc.vector.tensor_scalar(
                        n2e[t][s % RING][:], A_sb[t], C_sb[t][:, s : s + 1], 0.0,
                        op0=OP.add, op1=OP.max,
                    ).then_inc(v_sem, 1)
                if s % 2 == 1:
                    pr = (s - 3) // 2
                    if pr >= 0:
                        emit_ttm(vector, pr)
            emit_ttm(vector, NP - 1)
            vector.wait_ge(p_sem, P_LOOP)                    # all agg done
            for t in range(HT):
                nc.vector.tensor_copy(agg_sb[t][:], agg_ps[t][:]).then_inc(v_sem, 1)
            vector.wait_ge(pp_sem, 9)
            nc.vector.tensor_tensor(res[:], out5[:], ps_ls4, op=OP.subtract).then_inc(vv_sem, 1)

        @block.scalar
        def _(scalar):
            for it in range(NI):
                q = it % 2
                scalar.wait_ge(p_sem, p_py[it])
                for ho in range(HT):
                    if it >= RING:
                        # e2e slot WAR: msk pair op of (it-RING)//2 consumed it
                        mp = (v_mskp0 if ho == 0 else v_mskp1)[(it - RING) // 2]
                        scalar.wait_ge(v_sem, mp)
                    nc.scalar.activation(
                        e2e_ap(ho, it), py[ho][q][:], AF.Relu, bias=b2_sb[ho]
                    ).then_inc(a_sem, 1)
            # ---- node MLP activations ----
            for ho in range(HT):
                scalar.wait_ge(pp_sem, ho + 1)
                nc.scalar.activation(out1[ho][:], ps_mlp[ho][:], AF.Relu,
                                     bias=bn1_sb[ho]).then_inc(aa_sem, 1)
            for ho in range(HT):
                scalar.wait_ge(pp_sem, 2 + ho + 1)
                nc.scalar.activation(out2[ho][:], ps_mlp[ho][:], AF.Relu,
                                     bias=bn2_sb[ho]).then_inc(aa_sem, 1)
            for ho in range(HT):
                scalar.wait_ge(pp_sem, 4 + ho + 1)
                nc.scalar.activation(out4[ho][:], ps_mlp[ho][:], AF.Identity,
                                     bias=bo1_sb[ho]).then_inc(aa_sem, 1)
            scalar.wait_ge(pp_sem, 7)
            nc.scalar.activation(out5[:], ps5, AF.Identity, bias=bo_sb).then_inc(aa_sem, 1)
            nc.scalar.activation(ex[:], out5[:], AF.Exp).then_inc(aa_sem, 1)
            scalar.wait_ge(pp_sem, 8)
            nc.scalar.activation(ls[:], ps_sum, AF.Ln).then_inc(aa_sem, 1)

        @block.tensor
        def _(pe):
            def agg_mm(jt):
                pe.wait_ge(v_sem, v_mskp0[jt // 2])
                nc.tensor.matmul(
                    agg_ps[0][:], ident, msk_ap(0, jt),
                    start=(jt == 0), stop=(jt == NI - 1),
                ).then_inc(p_sem, 1)
                pe.wait_ge(v_sem, v_mskp1[jt // 2])
                nc.tensor.matmul(
                    agg_ps[1][:], ident, msk_ap(1, jt),
                    start=(jt == 0), stop=(jt == NI - 1),
                ).then_inc(p_sem, 1)

            for it in range(NI):
                q = it % 2
                pe.wait_ge(v_sem, v_n2e[it])                # n2e(it) ready
                if it >= 2:
                    pe.wait_ge(a_sem, a_relu1[it - 2])      # py[q] WAR
                for ho in range(HT):
                    nc.tensor.matmul(
                        py[ho][q][:], W2_sb[0][:, ho * 128 : (ho + 1) * 128],
                        n2e[0][it % RING][:], start=True, stop=False,
                    )
                    nc.tensor.matmul(
                        py[ho][q][:], W2_sb[1][:, ho * 128 : (ho + 1) * 128],
                        n2e[1][it % RING][:], start=False, stop=True,
                    ).then_inc(p_sem, 1)
                if it >= LAG:
                    agg_mm(it - LAG)
            for jt in range(NI - LAG, NI):
                agg_mm(jt)
            # ---- node MLP matmuls ----  pp_sem milestones:
            # 1,2: mlp1[ho]  3,4: mlp2[ho]  5,6: out4 ps[ho]  7: ps5
            # 8: ps_sum  9: ps_ls4
            pe.wait_ge(d_sem, D_AGGR)
            for ho in range(HT):
                for t in range(HT):
                    mm = nc.tensor.matmul(
                        ps_mlp[ho][:], Wn1_sb[t][:, ho * 128 : (ho + 1) * 128],
                        agg_sb[t][:], start=(t == 0), stop=(t == 1),
                    )
                mm.then_inc(pp_sem, 1)
            for ho in range(HT):
                pe.wait_ge(aa_sem, 2)
                for t in range(HT):
                    mm = nc.tensor.matmul(
                        ps_mlp[ho][:], Wn2_sb[t][:, ho * 128 : (ho + 1) * 128],
                        out1[t][:], start=(t == 0), stop=(t == 1),
                    )
                mm.then_inc(pp_sem, 1)
            for ho in range(HT):
                pe.wait_ge(aa_sem, 4)
                nc.tensor.matmul(
                    ps_mlp[ho][:], Wo1x_sb[:, ho * 128 : (ho + 1) * 128], xT_sb,
                    start=True, stop=False,
                )
                for t in range(HT):
                    mm = nc.tensor.matmul(
                        ps_mlp[ho][:], Wo1h_sb[t][:, ho * 128 : (ho + 1) * 128],
                        out2[t][:], start=False, stop=(t == 1),
                    )
                mm.then_inc(pp_sem, 1)
            pe.wait_ge(aa_sem, 6)
            for t in range(HT):
                mm = nc.tensor.matmul(
                    ps5, Wo_sb[t], out4[t][:], start=(t == 0), stop=(t == 1),
                )
            mm.then_inc(pp_sem, 1)
            pe.wait_ge(aa_sem, 8)                           # ex ready
            nc.tensor.matmul(ps_sum, ones4, ex[:], start=True, stop=True).then_inc(pp_sem, 1)
            pe.wait_ge(aa_sem, 9)                           # ls ready
            nc.tensor.matmul(ps_ls4, ones14, ls[:], start=True, stop=True).then_inc(pp_sem, 1)

    return nc


def make_in_maps(x, adj, W_e1, b_e1, W_e2, b_e2, W_n1, b_n1, W_n2, b_n2,
                 W_o1, b_o1, W_o, b_o):
    in_maps = []
    for c in range(8):
        b = c // 2
        i0 = (c % 2) * NI
        A_full = x[b] @ W_e1[:D]                     # [N, H]
        C_full = x[b] @ W_e1[D:] + b_e1              # [N, H]

        cf16 = np.zeros((128, CF16), np.float16)
        AT = A_full.T.astype(np.float16)             # [H, N]
        af16 = np.zeros((128, 2 * N), np.float16)
        for t in range(HT):
            af16[:, t * N : (t + 1) * N] = AT[t * 128 : (t + 1) * 128]
            cf16[:, O_W2 + t * H : O_W2 + (t + 1) * H] = \
                W_e2[t * 128 : (t + 1) * 128].astype(np.float16)
        cf16[:, O_ID : O_ID + 128] = np.eye(128, dtype=np.float16)

        cf32 = np.zeros((128, CF32), np.float32)
        CT = C_full[i0 : i0 + NI].T.astype(np.float32)   # [H, NI]
        for t in range(HT):
            r = slice(t * 128, (t + 1) * 128)
            cf32[:, O_C + t * NI : O_C + (t + 1) * NI] = CT[r]
            cf32[:, O_B2 + t] = b_e2[r]
            cf32[:, O_WN1 + t * H : O_WN1 + (t + 1) * H] = W_n1[r]
            cf32[:, O_WN2 + t * H : O_WN2 + (t + 1) * H] = W_n2[r]
            cf32[:, O_WO1H + t * H : O_WO1H + (t + 1) * H] = W_o1[D:][r]
            cf32[:, O_WO + t * D : O_WO + (t + 1) * D] = W_o[r]
            cf32[:, O_BN1 + t] = b_n1[r]
            cf32[:, O_BN2 + t] = b_n2[r]
            cf32[:, O_BO1 + t] = b_o1[r]
        cf32[0:D, O_XT : O_XT + N] = x[b].T
        cf32[0:D, O_WO1X : O_WO1X + H] = W_o1[:D]
        cf32[0:D, O_BO] = b_o
        cf32[0:D, O_ONES4] = 1.0
        cf32[0:1, O_ONES14 : O_ONES14 + 4] = 1.0

        in_maps.append({
            "cf16": cf16,
            "af16": af16,
            "cf32": cf32,
            "adjr": adj[b, i0 : i0 + NI, :].astype(np.float16),
        })
    return in_maps


def run(trace=False, **inputs):
    if "nc" not in _CACHE:
        _CACHE["nc"] = build_program()
    nc = _CACHE["nc"]
    in_maps = make_in_maps(**{k: np.asarray(v) for k, v in inputs.items()})
    r = run_bass_kernel_spmd(nc, in_maps, list(range(8)), trace=trace)
    out = np.stack([r.results[2 * b]["out"] for b in range(B)]).astype(np.float32)
    return out, r


def kernel(**inputs):
    out, _ = run(trace=False, **inputs)
    return out
